# revision 58
# baseline (speedup 1.0000x reference)
"""Channel-attention MultiHeadAttention kernel for Trainium2 (8 NeuronCores).

Math: attention is over channels (d x d per head) with the spatial dim
N = H*W as the contraction axis. The whole module collapses:
  G_aug = [x^T|1]^T [x^T|1] = [[G, s],[s^T, N]]  (257x257 Gram over N)
  S = scale * Qa_aug @ G_aug @ Ka_aug^T   (only 8 diag 32x32 blocks needed)
  attn = softmax(S_blocks)
  M1 = bd(attn)-as-lhsT @ WoutP^T ; WfT_aug = wva-as-lhsT @ M1
  out = Wf @ x + bf   (bias applied as rank-1 ones-row matmul)
Sharding: data-parallel over batch B=8, one batch element per core.

Perf design (CoreSim cost model: per-core DMA bus ~332 GB/s effective,
so 16 MB in + 16 MB out = ~101 us is the hard floor per element):
 - f32->f16 casts on the otherwise-idle Pool engine.
 - PE transposes software-pipelined ahead of the Gram matmuls (LAG=3).
 - phase C DMAs straight out of PSUM (no staging copies at all).
 - repeat>1 (the benchmark loop) runs a 2-element software pipeline:
   each For_i body streams element i's output while loading + Gram-ing
   element i+1 (double-buffered x16/wf16/bf16), with in/out DMAs woven
   on the SP queue so the DMA pool never idles; the serial Gram->softmax
   ->Wfinal chain hides entirely under the other element's DMA stream.
"""

import numpy as np
from contextlib import ExitStack

B, C, H, W = 8, 256, 128, 128
N = H * W          # 16384
NH, D = 8, 32      # heads, head dim
SCALE = D ** -0.5
SUB = 128          # transpose subchunk
NCORES = 8

ICH = 1024         # in-token width (cols, both channel halves)
OCH = 1024         # out-token width
NTOK = N // ICH    # 16
CPIECE = 1024      # cast piece width (Pool engine ops)
OSUB = 512         # phase C matmul free-dim (1 PSUM bank)
LAG = 3            # Gram matmuls lag transposes by this many subchunks

TRACE = False      # test.py may set kernel.TRACE = True
LAST_RESULTS = {}  # exec_time_ns etc. for test.py

_CACHE = {}


def _build_real(repeat=1):
    import concourse.bacc as bacc
    import concourse.mybir as mybir
    import concourse.tile as tile

    dt = mybir.dt
    f32, f16 = dt.float32, dt.float16
    Exp = mybir.ActivationFunctionType.Exp
    X = mybir.AxisListType.X

    nc = bacc.Bacc(trn_type="TRN2")

    x_d = nc.dram_tensor("xb", [C, N], f32, kind="ExternalInput")
    qaT_d = nc.dram_tensor("qaT", [257, 256], f16, kind="ExternalInput")
    kaT_d = nc.dram_tensor("kaT", [257, 256], f16, kind="ExternalInput")
    wva_d = nc.dram_tensor("wva", [256, 257], f16, kind="ExternalInput")
    wpT_d = nc.dram_tensor("wpT", [256, 256], f16, kind="ExternalInput")
    boutr_d = nc.dram_tensor("boutr", [1, 256], f32, kind="ExternalInput")
    qbr_d = nc.dram_tensor("qbr", [1, 256], f16, kind="ExternalInput")
    kbr_d = nc.dram_tensor("kbr", [1, 256], f16, kind="ExternalInput")
    id_d = nc.dram_tensor("ident", [128, 128], f16, kind="ExternalInput")
    gdA_d = nc.dram_tensor("gdA", [128, 257], f32, kind="ExternalInput")
    gdB_d = nc.dram_tensor("gdB", [128, 257], f32, kind="ExternalInput")
    corrh_d = nc.dram_tensor("corrh", [256, 256], f16, kind="ExternalInput")
    corrl_d = nc.dram_tensor("corrl", [256, 256], f16, kind="ExternalInput")
    out_d = nc.dram_tensor("out", [C, N], f32, kind="ExternalOutput")

    NPAR = 1 if repeat == 1 else 2

    with ExitStack() as top:
        tc = top.enter_context(tile.TileContext(nc))
        persist = top.enter_context(tc.tile_pool(name="persist", bufs=1))

        # double-buffered per-element state (parity = element mod 2)
        x16 = [[persist.tile([128, N], f16, tag=f"x16_{p}_{i}", name=f"x16_{p}_{i}")
                for i in range(2)] for p in range(NPAR)]
        wf16 = [[persist.tile([128, 256], f16, tag=f"wf_{p}_{k}", name=f"wf_{p}_{k}")
                 for k in range(2)] for p in range(NPAR)]
        bf16 = [persist.tile([1, 256], f16, tag=f"bf_{p}", name=f"bf_{p}")
                for p in range(NPAR)]

        qaT_t = [persist.tile([128, 256], f16, tag=f"qaT{k}", name=f"qaT{k}") for k in range(2)]
        kaT_t = [persist.tile([128, 256], f16, tag=f"kaT{k}", name=f"kaT{k}") for k in range(2)]
        qbr_t = persist.tile([1, 256], f16, tag="qbr", name="qbr")
        kbr_t = persist.tile([1, 256], f16, tag="kbr", name="kbr")
        wva_t = [persist.tile([128, 257], f16, tag=f"wva{k}", name=f"wva{k}") for k in range(2)]
        wpT_t = [persist.tile([128, 256], f16, tag=f"wpT{k}", name=f"wpT{k}") for k in range(2)]
        boutr_t = persist.tile([1, 256], f32, tag="boutr", name="boutr")
        id16 = persist.tile([128, 128], f16, tag="id16", name="id16")
        gd_t = [persist.tile([128, 257], f32, tag=f"gd{k}", name=f"gd{k}") for k in range(2)]
        corr_t = [[persist.tile([128, 256], f16, tag=f"corr{j}{q}", name=f"corr{j}{q}")
                   for q in range(2)] for j in range(2)]
        ones16 = persist.tile([1, OSUB], f16, tag="ones16", name="ones16")

        nc.scalar.dma_start(out=qaT_t[0], in_=qaT_d.ap()[0:128, :])
        nc.scalar.dma_start(out=qaT_t[1], in_=qaT_d.ap()[128:256, :])
        nc.scalar.dma_start(out=kaT_t[0], in_=kaT_d.ap()[0:128, :])
        nc.scalar.dma_start(out=kaT_t[1], in_=kaT_d.ap()[128:256, :])
        nc.scalar.dma_start(out=qbr_t, in_=qbr_d.ap())
        nc.scalar.dma_start(out=kbr_t, in_=kbr_d.ap())
        nc.scalar.dma_start(out=wva_t[0], in_=wva_d.ap()[0:128, :])
        nc.scalar.dma_start(out=wva_t[1], in_=wva_d.ap()[128:256, :])
        nc.scalar.dma_start(out=wpT_t[0], in_=wpT_d.ap()[0:128, :])
        nc.scalar.dma_start(out=wpT_t[1], in_=wpT_d.ap()[128:256, :])
        nc.scalar.dma_start(out=boutr_t, in_=boutr_d.ap())
        nc.scalar.dma_start(out=id16, in_=id_d.ap())
        nc.scalar.dma_start(out=gd_t[0], in_=gdA_d.ap())
        nc.scalar.dma_start(out=gd_t[1], in_=gdB_d.ap())
        for j, cd in enumerate((corrh_d, corrl_d)):
            nc.scalar.dma_start(out=corr_t[j][0], in_=cd.ap()[0:128, :])
            nc.scalar.dma_start(out=corr_t[j][1], in_=cd.ap()[128:256, :])
        nc.gpsimd.memset(ones16[:], 1.0)

        # pools live for the whole kernel (PSUM: 2+2+2+2 = 8 banks)
        psA = top.enter_context(tc.tile_pool(name="psA", bufs=1, space="PSUM"))
        psT = top.enter_context(tc.tile_pool(name="psT", bufs=2, space="PSUM"))
        pst = top.enter_context(tc.tile_pool(name="pst", bufs=2, space="PSUM"))
        psC = top.enter_context(tc.tile_pool(name="psC", bufs=2, space="PSUM"))
        stage = top.enter_context(tc.tile_pool(name="stage", bufs=2))
        ost = top.enter_context(tc.tile_pool(name="ost", bufs=2))
        tp = top.enter_context(tc.tile_pool(name="tinysb", bufs=1))
        cst = {"cnt": 0}

        G_ps = [psA.tile([128, 257], f32, tag="g0", name="g0"),
                psA.tile([128, 129], f32, tag="g1", name="g1")]

        NXT = 8
        xts = [persist.tile([128, 257], f16, tag=f"xt{j}", name=f"xt{j}")
               for j in range(NXT)]

        ast = {}  # per-A-phase emission state

        def a_begin(par):
            ast.clear()
            ast.update(par=par, t=0, pend=[])
            for j in range(NXT):
                nc.vector.memset(xts[j][:, 256:257], 1.0)

        def emit_g(ent, last):
            pxt, pfirst = ent
            nc.tensor.matmul(G_ps[0][:], lhsT=pxt[:, 0:128], rhs=pxt[:],
                             start=pfirst, stop=last)
            nc.tensor.matmul(G_ps[1][:], lhsT=pxt[:, 128:256],
                             rhs=pxt[:, 128:257], start=pfirst, stop=last)

        def a_token(k):
            par = ast["par"]
            n0 = k * ICH
            xs = [stage.tile([128, ICH], f32, tag=f"xs{i}", name=f"xs{i}")
                  for i in range(2)]
            sl = slice(n0, n0 + ICH)
            nc.sync.dma_start(out=xs[0][:], in_=x_d.ap()[0:128, sl])
            nc.sync.dma_start(out=xs[1][:], in_=x_d.ap()[128:256, sl])
            for p0 in range(0, ICH, CPIECE):
                for i in range(2):
                    nc.gpsimd.tensor_copy(
                        out=x16[par][i][:, n0 + p0:n0 + p0 + CPIECE],
                        in_=xs[i][:, p0:p0 + CPIECE])
            for ci in range(ICH // SUB):
                c0 = n0 + ci * SUB
                t = ast["t"]
                xt = xts[t % NXT]
                tp_ps = psT.tile([128, 256], f16, tag="tps", name="tps")
                nc.tensor.transpose(tp_ps[:, 0:128],
                                    x16[par][0][:, c0:c0 + SUB], id16[:])
                nc.tensor.transpose(tp_ps[:, 128:256],
                                    x16[par][1][:, c0:c0 + SUB], id16[:])
                if t % 2 == 0:
                    nc.vector.tensor_copy(out=xt[:, 0:256], in_=tp_ps[:])
                else:
                    nc.scalar.copy(out=xt[:, 0:256], in_=tp_ps[:])
                ast["pend"].append((xt, t == 0))
                if len(ast["pend"]) > LAG:
                    emit_g(ast["pend"].pop(0), False)
                ast["t"] = t + 1

        def a_finish():
            """Drain Gram pipeline, then the tiny stage -> wf16/bf16."""
            par = ast["par"]
            pend = ast["pend"]
            while pend:
                emit_g(pend.pop(0), not pend)

            # blockdiag(attn) background zeros
            abd = [tp.tile([128, 128], f16, tag=f"abd{q}", name=f"abd{q}")
                   for q in range(2)]
            for q in range(2):
                nc.gpsimd.memset(abd[q][:], 0.0)

            # s columns: plain f16 copies (no diag term in col 256)
            # (Pool cannot touch PSUM on HW: DVE + Act here)
            scol = [tp.tile([128, 1], f16, tag=f"scol{k}", name=f"scol{k}")
                    for k in range(2)]
            nc.vector.tensor_copy(out=scol[0][:], in_=G_ps[0][:, 256:257])
            nc.scalar.copy(out=scol[1][:], in_=G_ps[1][:, 128:129])

            # u^T = s^T Qm^T, v^T = s^T Km^T (folded into SF as rank-1s)
            uv = [tp.tile([1, 256], f16, tag=f"uv{j}", name=f"uv{j}")
                  for j in range(2)]
            for j, wt in enumerate((qaT_t, kaT_t)):
                uv_ps = pst.tile([1, 256], f32, tag="tinyps", name="tinyps")
                for k in range(2):
                    nc.tensor.matmul(uv_ps[:], lhsT=scol[k][:], rhs=wt[k][:],
                                     start=(k == 0), stop=(k == 1))
                if j == 0:
                    nc.scalar.copy(out=uv[j][:], in_=uv_ps[:])
                else:
                    nc.vector.tensor_copy(out=uv[j][:], in_=uv_ps[:])

            # Ga panels: [G00-NI | G01 | s0] and [G11-NI | s1], fp16
            Ga = [tp.tile([128, 257], f16, tag="Ga0", name="Ga0"),
                  tp.tile([128, 129], f16, tag="Ga1", name="Ga1")]
            nc.vector.tensor_sub(Ga[1][:], G_ps[1][:], gd_t[1][:, 0:129])
            nc.vector.tensor_sub(Ga[0][:], G_ps[0][:], gd_t[0][:])
            # G10 = G01^T via one PE transpose
            g10 = tp.tile([128, 128], f16, tag="g10", name="g10")
            g10_ps = pst.tile([128, 128], f16, tag="tinyps", name="tinyps")
            nc.tensor.transpose(g10_ps[:], Ga[0][:, 128:256], id16[:])
            nc.scalar.copy(out=g10[:], in_=g10_ps[:])

            # T2 = Gm @ Km^T; m=1 first (m=0 needs g10)
            t2s = [tp.tile([128, 256], f16, tag="t2s0", name="t2s0"),
                   tp.tile([128, 256], f16, tag="t2s1", name="t2s1")]
            t2lhs = [[Ga[0][:, 0:128], g10[:]],
                     [Ga[0][:, 128:256], Ga[1][:, 0:128]]]
            for m in (1, 0):
                t2_ps = pst.tile([128, 256], f32, tag="tinyps", name="tinyps")
                for k in range(2):
                    nc.tensor.matmul(t2_ps[:], lhsT=t2lhs[m][k], rhs=kaT_t[k][:],
                                     start=(k == 0), stop=(k == 1))
                if m == 1:
                    nc.scalar.copy(out=t2s[m][:], in_=t2_ps[:])
                else:
                    nc.vector.tensor_copy(out=t2s[m][:], in_=t2_ps[:])

            # S_full = corr (preloaded via identity matmuls, exact hi+lo f16
            # split) + Qm T2 + u (x) kb^T + qb (x) v^T; k=0 last (t2s0 late)
            SF = []
            for m in range(2):
                msl = slice(128 * m, 128 * (m + 1))
                sf_ps = pst.tile([128, 256], f32, tag="tinyps", name=f"sf{m}")
                nc.tensor.matmul(sf_ps[:], lhsT=id16[:], rhs=corr_t[0][m][:],
                                 start=True, stop=False)
                nc.tensor.matmul(sf_ps[:], lhsT=id16[:], rhs=corr_t[1][m][:],
                                 start=False, stop=False)
                nc.tensor.matmul(sf_ps[:], lhsT=qaT_t[1][:, msl],
                                 rhs=t2s[1][:], start=False, stop=False)
                nc.tensor.matmul(sf_ps[:], lhsT=uv[0][0:1, msl], rhs=kbr_t[:],
                                 start=False, stop=False)
                nc.tensor.matmul(sf_ps[:], lhsT=qbr_t[0:1, msl], rhs=uv[1][:],
                                 start=False, stop=False)
                nc.tensor.matmul(sf_ps[:], lhsT=qaT_t[0][:, msl],
                                 rhs=t2s[0][:], start=False, stop=True)
                SF.append(sf_ps)

            # extract diag blocks -> stacked [128, 32] per q-group (copies,
            # split DVE/Act; Pool cannot read PSUM)
            Sst = [tp.tile([128, 32], f32, tag=f"sstk{q}", name=f"sstk{q}")
                   for q in range(2)]
            for h in range(NH):
                q, po = h // 4, (h % 4) * 32
                eng = nc.vector if h % 2 == 0 else nc.scalar
                if eng is nc.vector:
                    eng.tensor_copy(out=Sst[q][po:po + 32, :],
                                    in_=SF[q][po:po + 32, h * 32:(h + 1) * 32])
                else:
                    eng.copy(out=Sst[q][po:po + 32, :],
                             in_=SF[q][po:po + 32, h * 32:(h + 1) * 32])

            # softmax over free dim; fused exp+sum; scale writes straight
            # into the block-diagonal layout
            for q in range(2):
                nm = tp.tile([128, 1], f32, tag=f"nm{q}", name=f"nm{q}")
                nc.vector.reduce_max(out=nm[:], in_=Sst[q][:], axis=X, negate=True)
                ex = tp.tile([128, 32], f32, tag=f"ex{q}", name=f"ex{q}")
                sm = tp.tile([128, 1], f32, tag=f"sm{q}", name=f"sm{q}")
                nc.scalar.activation(out=ex[:], in_=Sst[q][:], func=Exp,
                                     bias=nm[:], scale=1.0, accum_out=sm[:])
                rc = tp.tile([128, 1], f32, tag=f"rc{q}", name=f"rc{q}")
                nc.vector.reciprocal(out=rc[:], in_=sm[:])
                for hh in range(4):
                    po = hh * 32
                    eng = nc.vector if hh % 2 == 0 else nc.gpsimd
                    eng.tensor_scalar_mul(abd[q][po:po + 32, po:po + 32],
                                          ex[po:po + 32, :], rc[po:po + 32, :])

            # M1[c, co] = sum_k bd(A)[k, c] * WoutP^T[k, co]
            m1s = [tp.tile([128, 256], f16, tag=f"m1s{q}", name=f"m1s{q}")
                   for q in range(2)]
            for q in range(2):
                m1_ps = pst.tile([128, 256], f32, tag="tinyps", name="tinyps")
                nc.tensor.matmul(m1_ps[:], lhsT=abd[q][:], rhs=wpT_t[q][:],
                                 start=True, stop=True)
                if q == 0:
                    nc.scalar.copy(out=m1s[q][:], in_=m1_ps[:])
                else:
                    nc.vector.tensor_copy(out=m1s[q][:], in_=m1_ps[:])

            # WfT_aug[c2, co] = sum_c wva[c, c2] * M1[c, co]; row 256 = bias
            for m in range(2):
                msl = slice(128 * m, 128 * (m + 1))
                wf_ps = pst.tile([128, 256], f32, tag="tinyps", name=f"wfps{m}")
                for k in range(2):
                    nc.tensor.matmul(wf_ps[:], lhsT=wva_t[k][:, msl],
                                     rhs=m1s[k][:], start=(k == 0), stop=(k == 1))
                if m == 1:
                    nc.scalar.copy(out=wf16[par][m][:], in_=wf_ps[:])
                else:
                    nc.vector.tensor_copy(out=wf16[par][m][:], in_=wf_ps[:])
            bf_ps = pst.tile([1, 256], f32, tag="tinyps", name="tinyps")
            for k in range(2):
                nc.tensor.matmul(bf_ps[:], lhsT=wva_t[k][:, 256:257],
                                 rhs=m1s[k][:], start=(k == 0), stop=(k == 1))
            nc.vector.tensor_add(bf16[par][:], bf_ps[:], boutr_t[:])

        def c_token(par, k):
            """Stream OCH output cols: matmuls into PSUM, copy, DMA out."""
            n0 = k * OCH
            for m in range(2):
                msl = slice(128 * m, 128 * (m + 1))
                o_sb = ost.tile([128, OCH], f32, tag=f"osb{m}", name=f"osb{m}")
                for h0 in range(0, OCH, OSUB):
                    sl = slice(n0 + h0, n0 + h0 + OSUB)
                    o_ps = psC.tile([128, OSUB], f32, tag="ops", name="ops")
                    nc.tensor.matmul(o_ps[:], lhsT=wf16[par][0][:, msl],
                                     rhs=x16[par][0][:, sl], start=True, stop=False)
                    nc.tensor.matmul(o_ps[:], lhsT=wf16[par][1][:, msl],
                                     rhs=x16[par][1][:, sl], start=False, stop=False)
                    nc.tensor.matmul(o_ps[:], lhsT=bf16[par][0:1, msl],
                                     rhs=ones16[:], start=False, stop=True)
                    dst = o_sb[:, h0:h0 + OSUB]
                    r = cst["cnt"] % 2
                    cst["cnt"] += 1
                    if r == 0:
                        nc.vector.tensor_copy(out=dst, in_=o_ps[:])
                    else:
                        nc.scalar.copy(out=dst, in_=o_ps[:])
                nc.sync.dma_start(out=out_d.ap()[msl, n0:n0 + OCH], in_=o_sb[:])

        def pipelined_half(pc, pa):
            """C(pc) woven with A(pa): in-DMAs lead so the Gram+softmax+
            Wfinal chain of pa completes before pc's stream ends."""
            ai = ci = 0
            a_begin(pa)
            sched = []
            for _ in range(8):
                sched += ["A", "C", "A"]
            sched += ["C"] * 8
            for tok in sched:
                if tok == "A":
                    a_token(ai)
                    ai += 1
                    if ai == NTOK:
                        a_finish()
                else:
                    c_token(pc, ci)
                    ci += 1

        if repeat == 1:
            a_begin(0)
            for k in range(NTOK):
                a_token(k)
            a_finish()
            for k in range(NTOK):
                c_token(0, k)
        else:
            a_begin(0)
            for k in range(NTOK):
                a_token(k)
            a_finish()
            with tc.For_i(0, repeat, 1):
                pipelined_half(0, 1)
                pipelined_half(1, 0)

    nc.finalize()
    return nc


def _host_prep(Wqkv, bqkv, Wout, bout):
    Wq, Wk, Wv = Wqkv[:C], Wqkv[C:2 * C], Wqkv[2 * C:]
    bq, bk, bv = bqkv[:C], bqkv[C:2 * C], bqkv[2 * C:]
    qa = np.concatenate([Wq, bq[:, None]], axis=1) * SCALE      # (256, 257)
    ka = np.concatenate([Wk, bk[:, None]], axis=1)              # (256, 257)
    qaT = np.ascontiguousarray(qa.T)                            # (257, 256)
    kaT = np.ascontiguousarray(ka.T)
    wva = np.concatenate([Wv, bv[:, None]], axis=1)             # (256, 257)
    r = np.arange(C)
    WoutP = Wout[:, (r % D) * NH + (r // D)]                    # (256, 256)
    wpT = np.ascontiguousarray(WoutP.T)
    gdA = np.zeros((128, 257), dtype=np.float32)
    gdA[np.arange(128), np.arange(128)] = float(N)
    gdB = np.zeros((128, 257), dtype=np.float32)
    gdB[np.arange(128), np.arange(128)] = float(N)
    corr_full = float(N) * (qa @ ka.T)                          # (256, 256) fp32
    corrb = np.zeros((256, 256), dtype=np.float32)
    for h in range(NH):
        corrb[h * D:(h + 1) * D, h * D:(h + 1) * D] = \
            corr_full[h * D:(h + 1) * D, h * D:(h + 1) * D]
    corrh = corrb.astype(np.float16)
    corrl = (corrb - corrh.astype(np.float32)).astype(np.float16)
    return {
        "qaT": qaT.astype(np.float16), "kaT": kaT.astype(np.float16),
        "wva": np.ascontiguousarray(wva, dtype=np.float16),
        "wpT": wpT.astype(np.float16),
        "boutr": np.ascontiguousarray(bout[None, :], dtype=np.float32),
        "qbr": np.ascontiguousarray(bq[None, :] * SCALE, dtype=np.float16),
        "kbr": np.ascontiguousarray(bk[None, :], dtype=np.float16),
        "ident": np.eye(128, dtype=np.float16),
        "gdA": gdA, "gdB": gdB, "corrh": corrh, "corrl": corrl,
    }


def kernel(x, Wqkv, bqkv, Wout, bout, num_heads):
    from concourse.bass_utils import run_bass_kernel_spmd

    assert int(num_heads) == NH
    x = np.ascontiguousarray(np.asarray(x), dtype=np.float32)
    shared = _host_prep(
        np.asarray(Wqkv, dtype=np.float32), np.asarray(bqkv, dtype=np.float32),
        np.asarray(Wout, dtype=np.float32), np.asarray(bout, dtype=np.float32))

    if "nc" not in _CACHE:
        _CACHE["nc"] = _build_real()
    nc = _CACHE["nc"]

    in_maps = [{"xb": np.ascontiguousarray(x[c].reshape(C, N)), **shared}
               for c in range(NCORES)]

    res = run_bass_kernel_spmd(nc, in_maps, core_ids=list(range(NCORES)),
                               trace=TRACE)
    LAST_RESULTS["exec_time_ns"] = res.exec_time_ns
    out = np.stack([res.results[c]["out"] for c in range(NCORES)])
    return out.reshape(B, C, H, W)


# revision 79
# speedup vs baseline: 1.5498x; 1.5498x over previous
"""Channel-attention MultiHeadAttention kernel for Trainium2 (8 NeuronCores).

Math: attention is over channels (d x d per head) with the spatial dim
N = H*W as the contraction axis. The whole module collapses:
  G_aug = [x^T|1]^T [x^T|1] = [[G, s],[s^T, N]]  (257x257 Gram over N)
  S = scale * Qa_aug @ G_aug @ Ka_aug^T   (only 8 diag 32x32 blocks needed)
  attn = softmax(S_blocks)
  M1 = bd(attn)-as-lhsT @ WoutP^T ; WfT_aug = wva-as-lhsT @ M1
  out = Wf @ x + bf   (bias applied as rank-1 ones-row matmul)
Sharding: data-parallel over batch B=8, one batch element per core.

Perf design (CoreSim cost model: per-core DMA bus ~332 GB/s effective,
so 16 MB in + 16 MB out = ~101 us is the hard floor per element):
 - f32->f16 casts on the otherwise-idle Pool engine.
 - PE transposes software-pipelined ahead of the Gram matmuls (LAG=3).
 - phase C DMAs straight out of PSUM (no staging copies at all).
 - repeat>1 (the benchmark loop) runs a 2-element software pipeline:
   each For_i body streams element i's output while loading + Gram-ing
   element i+1 (double-buffered x16/wf16/bf16), with in/out DMAs woven
   on the SP queue so the DMA pool never idles; the serial Gram->softmax
   ->Wfinal chain hides entirely under the other element's DMA stream.
"""

import numpy as np
from contextlib import ExitStack

B, C, H, W = 8, 256, 128, 128
N = H * W          # 16384
NH, D = 8, 32      # heads, head dim
SCALE = D ** -0.5
SUB = 128          # transpose subchunk
NCORES = 8

ICH = 1024         # in-token width (cols, both channel halves)
OCH = 1024         # out-token width
NTOK = N // ICH    # 16
CPIECE = 1024      # cast piece width (Pool engine ops)
OSUB = 512         # phase C matmul free-dim (1 PSUM bank)
LAG = 3            # Gram matmuls lag transposes by this many subchunks

TRACE = False      # test.py may set kernel.TRACE = True
LAST_RESULTS = {}  # exec_time_ns etc. for test.py

_CACHE = {}


def _build_real(repeat=1, mode="pipe"):
    """mode: 'pipe' = 2-element software pipeline per For_i body;
    'serial' = one element per For_i body (A -> tiny -> C)."""
    import concourse.bacc as bacc
    import concourse.mybir as mybir
    import concourse.tile as tile

    dt = mybir.dt
    f32, f16 = dt.float32, dt.float16
    Exp = mybir.ActivationFunctionType.Exp
    X = mybir.AxisListType.X

    nc = bacc.Bacc(trn_type="TRN2")

    x_d = nc.dram_tensor("xb", [C, N], f32, kind="ExternalInput")
    qaT_d = nc.dram_tensor("qaT", [257, 256], f16, kind="ExternalInput")
    kaT_d = nc.dram_tensor("kaT", [257, 256], f16, kind="ExternalInput")
    wva_d = nc.dram_tensor("wva", [256, 257], f16, kind="ExternalInput")
    wpT_d = nc.dram_tensor("wpT", [256, 256], f16, kind="ExternalInput")
    boutc_d = nc.dram_tensor("boutc", [256, 1], f32, kind="ExternalInput")
    bvc_d = nc.dram_tensor("bvc", [256, 1], f16, kind="ExternalInput")
    qbr_d = nc.dram_tensor("qbr", [1, 256], f16, kind="ExternalInput")
    kbr_d = nc.dram_tensor("kbr", [1, 256], f16, kind="ExternalInput")
    id_d = nc.dram_tensor("ident", [128, 128], f16, kind="ExternalInput")
    gdA_d = nc.dram_tensor("gdA", [128, 257], f32, kind="ExternalInput")
    gdB_d = nc.dram_tensor("gdB", [128, 257], f32, kind="ExternalInput")
    corrh_d = nc.dram_tensor("corrh", [256, 256], f16, kind="ExternalInput")
    corrl_d = nc.dram_tensor("corrl", [256, 256], f16, kind="ExternalInput")
    out_d = nc.dram_tensor("out", [C, N], f32, kind="ExternalOutput")

    NPAR = 1 if (repeat == 1 or mode == "serial") else 2

    with ExitStack() as top:
        tc = top.enter_context(tile.TileContext(nc))
        persist = top.enter_context(tc.tile_pool(name="persist", bufs=1))

        # double-buffered per-element state (parity = element mod 2)
        x16 = [[persist.tile([128, N], f16, tag=f"x16_{p}_{i}", name=f"x16_{p}_{i}")
                for i in range(2)] for p in range(NPAR)]
        wf16 = [[persist.tile([128, 256], f16, tag=f"wf_{p}_{k}", name=f"wf_{p}_{k}")
                 for k in range(2)] for p in range(NPAR)]
        bfc = [[persist.tile([128, 1], f32, tag=f"bf_{p}_{m}", name=f"bf_{p}_{m}")
                for m in range(2)] for p in range(NPAR)]

        qaT_t = [persist.tile([128, 256], f16, tag=f"qaT{k}", name=f"qaT{k}") for k in range(2)]
        kaT_t = [persist.tile([128, 256], f16, tag=f"kaT{k}", name=f"kaT{k}") for k in range(2)]
        qbr_t = persist.tile([1, 256], f16, tag="qbr", name="qbr")
        kbr_t = persist.tile([1, 256], f16, tag="kbr", name="kbr")
        wva_t = [persist.tile([128, 257], f16, tag=f"wva{k}", name=f"wva{k}") for k in range(2)]
        wpT_t = [persist.tile([128, 256], f16, tag=f"wpT{k}", name=f"wpT{k}") for k in range(2)]
        boutc_t = [persist.tile([128, 1], f32, tag=f"boutc{m}", name=f"boutc{m}")
                   for m in range(2)]
        bvc_t = [persist.tile([128, 1], f16, tag=f"bvc{k}", name=f"bvc{k}")
                 for k in range(2)]
        id16 = persist.tile([128, 128], f16, tag="id16", name="id16")
        gd_t = [persist.tile([128, 257], f32, tag=f"gd{k}", name=f"gd{k}") for k in range(2)]
        corr_t = [[persist.tile([128, 256], f16, tag=f"corr{j}{q}", name=f"corr{j}{q}")
                   for q in range(2)] for j in range(2)]

        nc.scalar.dma_start(out=qaT_t[0], in_=qaT_d.ap()[0:128, :])
        nc.scalar.dma_start(out=qaT_t[1], in_=qaT_d.ap()[128:256, :])
        nc.scalar.dma_start(out=kaT_t[0], in_=kaT_d.ap()[0:128, :])
        nc.scalar.dma_start(out=kaT_t[1], in_=kaT_d.ap()[128:256, :])
        nc.scalar.dma_start(out=qbr_t, in_=qbr_d.ap())
        nc.scalar.dma_start(out=kbr_t, in_=kbr_d.ap())
        nc.scalar.dma_start(out=wva_t[0], in_=wva_d.ap()[0:128, :])
        nc.scalar.dma_start(out=wva_t[1], in_=wva_d.ap()[128:256, :])
        nc.scalar.dma_start(out=wpT_t[0], in_=wpT_d.ap()[0:128, :])
        nc.scalar.dma_start(out=wpT_t[1], in_=wpT_d.ap()[128:256, :])
        nc.scalar.dma_start(out=boutc_t[0], in_=boutc_d.ap()[0:128, :])
        nc.scalar.dma_start(out=boutc_t[1], in_=boutc_d.ap()[128:256, :])
        nc.scalar.dma_start(out=bvc_t[0], in_=bvc_d.ap()[0:128, :])
        nc.scalar.dma_start(out=bvc_t[1], in_=bvc_d.ap()[128:256, :])
        nc.scalar.dma_start(out=id16, in_=id_d.ap())
        nc.scalar.dma_start(out=gd_t[0], in_=gdA_d.ap())
        nc.scalar.dma_start(out=gd_t[1], in_=gdB_d.ap())
        for j, cd in enumerate((corrh_d, corrl_d)):
            nc.scalar.dma_start(out=corr_t[j][0], in_=cd.ap()[0:128, :])
            nc.scalar.dma_start(out=corr_t[j][1], in_=cd.ap()[128:256, :])

        # pools live for the whole kernel (PSUM banks: 2+2+1+3 = 8)
        psA = top.enter_context(tc.tile_pool(name="psA", bufs=1, space="PSUM"))
        psT = top.enter_context(tc.tile_pool(name="psT", bufs=2, space="PSUM"))
        pst = top.enter_context(tc.tile_pool(name="pst", bufs=1, space="PSUM"))
        psC = top.enter_context(tc.tile_pool(name="psC", bufs=3, space="PSUM"))
        stage = top.enter_context(tc.tile_pool(name="stage", bufs=3))
        ost = top.enter_context(tc.tile_pool(name="ost", bufs=3))
        tp = top.enter_context(tc.tile_pool(name="tinysb", bufs=1))
        cst = {"cnt": 0, "dmaq": []}

        G_ps = [psA.tile([128, 257], f32, tag="g0", name="g0"),
                psA.tile([128, 129], f32, tag="g1", name="g1")]

        NXT = 8
        xts = [persist.tile([128, 257], f16, tag=f"xt{j}", name=f"xt{j}")
               for j in range(NXT)]

        ast = {}  # per-A-phase emission state

        def a_begin(par):
            ast.clear()
            ast.update(par=par, t=0, pend=[])
            for j in range(NXT):
                nc.vector.memset(xts[j][:, 256:257], 1.0)

        def emit_g(ent, last):
            pxt, pfirst = ent
            nc.tensor.matmul(G_ps[0][:], lhsT=pxt[:, 0:128], rhs=pxt[:],
                             start=pfirst, stop=last)
            nc.tensor.matmul(G_ps[1][:], lhsT=pxt[:, 128:256],
                             rhs=pxt[:, 128:257], start=pfirst, stop=last)

        def a_token(k):
            par = ast["par"]
            n0 = k * ICH
            xs = [stage.tile([128, ICH], f32, tag=f"xs{i}", name=f"xs{i}")
                  for i in range(2)]
            sl = slice(n0, n0 + ICH)
            nc.sync.dma_start(out=xs[0][:], in_=x_d.ap()[0:128, sl])
            nc.sync.dma_start(out=xs[1][:], in_=x_d.ap()[128:256, sl])
            # f32 -> f16 casts: DVE + Act (Pool's real copy throughput is
            # far below the cost model's; measured ~3x slower on HW)
            for p0 in range(0, ICH, CPIECE):
                for i in range(2):
                    dst = x16[par][i][:, n0 + p0:n0 + p0 + CPIECE]
                    src = xs[i][:, p0:p0 + CPIECE]
                    if i == 0:
                        nc.vector.tensor_copy(out=dst, in_=src)
                    else:
                        nc.scalar.copy(out=dst, in_=src)
            for ci in range(ICH // SUB):
                c0 = n0 + ci * SUB
                t = ast["t"]
                xt = xts[t % NXT]
                tp_ps = psT.tile([128, 256], f16, tag="tps", name="tps")
                nc.tensor.transpose(tp_ps[:, 0:128],
                                    x16[par][0][:, c0:c0 + SUB], id16[:])
                nc.tensor.transpose(tp_ps[:, 128:256],
                                    x16[par][1][:, c0:c0 + SUB], id16[:])
                if t % 2 == 0:
                    nc.vector.tensor_copy(out=xt[:, 0:256], in_=tp_ps[:])
                else:
                    nc.scalar.copy(out=xt[:, 0:256], in_=tp_ps[:])
                ast["pend"].append((xt, t == 0))
                if len(ast["pend"]) > LAG:
                    emit_g(ast["pend"].pop(0), False)
                ast["t"] = t + 1

        def a_finish():
            """Drain Gram pipeline, then the tiny stage -> wf16/bf16."""
            par = ast["par"]
            pend = ast["pend"]
            while pend:
                emit_g(pend.pop(0), not pend)

            # blockdiag(attn) background zeros
            abd = [tp.tile([128, 128], f16, tag=f"abd{q}", name=f"abd{q}")
                   for q in range(2)]
            for q in range(2):
                nc.gpsimd.memset(abd[q][:], 0.0)

            # s columns: plain f16 copies (no diag term in col 256)
            # (Pool cannot touch PSUM on HW: DVE + Act here)
            scol = [tp.tile([128, 1], f16, tag=f"scol{k}", name=f"scol{k}")
                    for k in range(2)]
            nc.vector.tensor_copy(out=scol[0][:], in_=G_ps[0][:, 256:257])
            nc.scalar.copy(out=scol[1][:], in_=G_ps[1][:, 128:129])

            # u^T = s^T Qm^T, v^T = s^T Km^T (folded into SF as rank-1s)
            uv = [tp.tile([1, 256], f16, tag=f"uv{j}", name=f"uv{j}")
                  for j in range(2)]
            for j, wt in enumerate((qaT_t, kaT_t)):
                uv_ps = pst.tile([1, 256], f32, tag="tinyps", name="tinyps")
                for k in range(2):
                    nc.tensor.matmul(uv_ps[:], lhsT=scol[k][:], rhs=wt[k][:],
                                     start=(k == 0), stop=(k == 1))
                if j == 0:
                    nc.scalar.copy(out=uv[j][:], in_=uv_ps[:])
                else:
                    nc.vector.tensor_copy(out=uv[j][:], in_=uv_ps[:])

            # Ga panels: [G00-NI | G01 | s0] and [G11-NI | s1], fp16
            Ga = [tp.tile([128, 257], f16, tag="Ga0", name="Ga0"),
                  tp.tile([128, 129], f16, tag="Ga1", name="Ga1")]
            nc.vector.tensor_sub(Ga[1][:], G_ps[1][:], gd_t[1][:, 0:129])
            nc.vector.tensor_sub(Ga[0][:], G_ps[0][:], gd_t[0][:])
            # G10 = G01^T via one PE transpose
            g10 = tp.tile([128, 128], f16, tag="g10", name="g10")
            g10_ps = pst.tile([128, 128], f16, tag="tinyps", name="tinyps")
            nc.tensor.transpose(g10_ps[:], Ga[0][:, 128:256], id16[:])
            nc.scalar.copy(out=g10[:], in_=g10_ps[:])

            # T2 = Gm @ Km^T; m=1 first (m=0 needs g10)
            t2s = [tp.tile([128, 256], f16, tag="t2s0", name="t2s0"),
                   tp.tile([128, 256], f16, tag="t2s1", name="t2s1")]
            t2lhs = [[Ga[0][:, 0:128], g10[:]],
                     [Ga[0][:, 128:256], Ga[1][:, 0:128]]]
            for m in (1, 0):
                t2_ps = pst.tile([128, 256], f32, tag="tinyps", name="tinyps")
                for k in range(2):
                    nc.tensor.matmul(t2_ps[:], lhsT=t2lhs[m][k], rhs=kaT_t[k][:],
                                     start=(k == 0), stop=(k == 1))
                if m == 1:
                    nc.scalar.copy(out=t2s[m][:], in_=t2_ps[:])
                else:
                    nc.vector.tensor_copy(out=t2s[m][:], in_=t2_ps[:])

            # S_full = corr (preloaded via identity matmuls, exact hi+lo f16
            # split) + Qm T2 + u (x) kb^T + qb (x) v^T; k=0 last (t2s0 late)
            SF = []
            for m in range(2):
                msl = slice(128 * m, 128 * (m + 1))
                sf_ps = pst.tile([128, 256], f32, tag="tinyps", name=f"sf{m}")
                nc.tensor.matmul(sf_ps[:], lhsT=id16[:], rhs=corr_t[0][m][:],
                                 start=True, stop=False)
                nc.tensor.matmul(sf_ps[:], lhsT=id16[:], rhs=corr_t[1][m][:],
                                 start=False, stop=False)
                nc.tensor.matmul(sf_ps[:], lhsT=qaT_t[1][:, msl],
                                 rhs=t2s[1][:], start=False, stop=False)
                nc.tensor.matmul(sf_ps[:], lhsT=uv[0][0:1, msl], rhs=kbr_t[:],
                                 start=False, stop=False)
                nc.tensor.matmul(sf_ps[:], lhsT=qbr_t[0:1, msl], rhs=uv[1][:],
                                 start=False, stop=False)
                nc.tensor.matmul(sf_ps[:], lhsT=qaT_t[0][:, msl],
                                 rhs=t2s[0][:], start=False, stop=True)
                SF.append(sf_ps)

            # extract diag blocks -> stacked [128, 32] per q-group (copies,
            # split DVE/Act; Pool cannot read PSUM)
            Sst = [tp.tile([128, 32], f32, tag=f"sstk{q}", name=f"sstk{q}")
                   for q in range(2)]
            for h in range(NH):
                q, po = h // 4, (h % 4) * 32
                eng = nc.vector if h % 2 == 0 else nc.scalar
                if eng is nc.vector:
                    eng.tensor_copy(out=Sst[q][po:po + 32, :],
                                    in_=SF[q][po:po + 32, h * 32:(h + 1) * 32])
                else:
                    eng.copy(out=Sst[q][po:po + 32, :],
                             in_=SF[q][po:po + 32, h * 32:(h + 1) * 32])

            # softmax over free dim; fused exp+sum; scale writes straight
            # into the block-diagonal layout
            for q in range(2):
                nm = tp.tile([128, 1], f32, tag=f"nm{q}", name=f"nm{q}")
                nc.vector.reduce_max(out=nm[:], in_=Sst[q][:], axis=X, negate=True)
                ex = tp.tile([128, 32], f32, tag=f"ex{q}", name=f"ex{q}")
                sm = tp.tile([128, 1], f32, tag=f"sm{q}", name=f"sm{q}")
                nc.scalar.activation(out=ex[:], in_=Sst[q][:], func=Exp,
                                     bias=nm[:], scale=1.0, accum_out=sm[:])
                rc = tp.tile([128, 1], f32, tag=f"rc{q}", name=f"rc{q}")
                nc.vector.reciprocal(out=rc[:], in_=sm[:])
                for hh in range(4):
                    po = hh * 32
                    eng = nc.vector if hh % 2 == 0 else nc.gpsimd
                    eng.tensor_scalar_mul(abd[q][po:po + 32, po:po + 32],
                                          ex[po:po + 32, :], rc[po:po + 32, :])

            # M1[c, co] = sum_k bd(A)[k, c] * WoutP^T[k, co]
            m1s = [tp.tile([128, 256], f16, tag=f"m1s{q}", name=f"m1s{q}")
                   for q in range(2)]
            for q in range(2):
                m1_ps = pst.tile([128, 256], f32, tag="tinyps", name="tinyps")
                nc.tensor.matmul(m1_ps[:], lhsT=abd[q][:], rhs=wpT_t[q][:],
                                 start=True, stop=True)
                if q == 0:
                    nc.scalar.copy(out=m1s[q][:], in_=m1_ps[:])
                else:
                    nc.vector.tensor_copy(out=m1s[q][:], in_=m1_ps[:])

            # WfT_aug[c2, co] = sum_c wva[c, c2] * M1[c, co]; row 256 = bias
            for m in range(2):
                msl = slice(128 * m, 128 * (m + 1))
                wf_ps = pst.tile([128, 256], f32, tag="tinyps", name=f"wfps{m}")
                for k in range(2):
                    nc.tensor.matmul(wf_ps[:], lhsT=wva_t[k][:, msl],
                                     rhs=m1s[k][:], start=(k == 0), stop=(k == 1))
                if m == 1:
                    nc.scalar.copy(out=wf16[par][m][:], in_=wf_ps[:])
                else:
                    nc.vector.tensor_copy(out=wf16[par][m][:], in_=wf_ps[:])
            # bias columns: bf[co] = sum_c bv[c] M1[c, co] + bout[co], with
            # M1 itself as lhsT (free-dim-1 matmuls, nearly free)
            for m in range(2):
                msl = slice(128 * m, 128 * (m + 1))
                bf_ps = pst.tile([128, 1], f32, tag="tinyps", name="tinyps")
                for k in range(2):
                    nc.tensor.matmul(bf_ps[:], lhsT=m1s[k][:, msl],
                                     rhs=bvc_t[k][:], start=(k == 0), stop=(k == 1))
                nc.vector.tensor_add(bfc[par][m][:], bf_ps[:], boutc_t[m][:])

        def c_token(par, k, lag=0):
            """Stream OCH output cols: matmuls into PSUM, copy to SBUF; the
            out-DMA is (optionally) emitted `lag` tokens late so the SP
            queue never head-of-line blocks on unproduced data."""
            n0 = k * OCH
            for m in range(2):
                msl = slice(128 * m, 128 * (m + 1))
                o_sb = ost.tile([128, OCH], f32, tag=f"osb{m}", name=f"osb{m}")
                # group matmuls by lhsT so the PE loads each weight once
                # per token-half (Ldweights is expensive on HW)
                subs = list(range(0, OCH, OSUB))
                o_pss = {}
                for h0 in subs:
                    o_pss[h0] = psC.tile([128, OSUB], f32, tag="ops", name="ops")
                for k in range(2):
                    for h0 in subs:
                        sl = slice(n0 + h0, n0 + h0 + OSUB)
                        nc.tensor.matmul(o_pss[h0][:], lhsT=wf16[par][k][:, msl],
                                         rhs=x16[par][k][:, sl],
                                         start=(k == 0), stop=(k == 1))
                for h0 in subs:
                    dst = o_sb[:, h0:h0 + OSUB]
                    r = cst["cnt"] % 2
                    cst["cnt"] += 1
                    if r == 0:
                        nc.vector.tensor_scalar_add(dst, o_pss[h0][:], bfc[par][m][:])
                    else:
                        nc.scalar.activation(
                            out=dst, in_=o_pss[h0][:],
                            func=mybir.ActivationFunctionType.Identity,
                            bias=bfc[par][m][:], scale=1.0)
                cst["dmaq"].append((msl, slice(n0, n0 + OCH), o_sb))
            while len(cst["dmaq"]) > 2 * lag:
                msl_, sl_, sb_ = cst["dmaq"].pop(0)
                nc.sync.dma_start(out=out_d.ap()[msl_, sl_], in_=sb_[:])

        def c_flush():
            while cst["dmaq"]:
                msl_, sl_, sb_ = cst["dmaq"].pop(0)
                nc.sync.dma_start(out=out_d.ap()[msl_, sl_], in_=sb_[:])

        def pipelined_half(pc, pa):
            """C(pc) woven with A(pa): in-DMAs lead so the Gram+softmax+
            Wfinal chain of pa completes before pc's stream ends."""
            ai = ci = 0
            a_begin(pa)
            sched = []
            for _ in range(8):
                sched += ["A", "C", "A"]
            sched += ["C"] * 8
            for tok in sched:
                if tok == "A":
                    a_token(ai)
                    ai += 1
                    if ai == NTOK:
                        a_finish()
                else:
                    c_token(pc, ci, lag=1)
                    ci += 1
            c_flush()

        def serial_element():
            a_begin(0)
            for k in range(NTOK):
                a_token(k)
            a_finish()
            for k in range(NTOK):
                c_token(0, k)

        if repeat == 1:
            serial_element()
        elif mode == "serial":
            with tc.For_i(0, repeat, 1):
                serial_element()
        else:
            a_begin(0)
            for k in range(NTOK):
                a_token(k)
            a_finish()
            with tc.For_i(0, repeat, 1):
                pipelined_half(0, 1)
                pipelined_half(1, 0)

    nc.finalize()
    return nc


def _host_prep(Wqkv, bqkv, Wout, bout):
    Wq, Wk, Wv = Wqkv[:C], Wqkv[C:2 * C], Wqkv[2 * C:]
    bq, bk, bv = bqkv[:C], bqkv[C:2 * C], bqkv[2 * C:]
    qa = np.concatenate([Wq, bq[:, None]], axis=1) * SCALE      # (256, 257)
    ka = np.concatenate([Wk, bk[:, None]], axis=1)              # (256, 257)
    qaT = np.ascontiguousarray(qa.T)                            # (257, 256)
    kaT = np.ascontiguousarray(ka.T)
    wva = np.concatenate([Wv, bv[:, None]], axis=1)             # (256, 257)
    r = np.arange(C)
    WoutP = Wout[:, (r % D) * NH + (r // D)]                    # (256, 256)
    wpT = np.ascontiguousarray(WoutP.T)
    gdA = np.zeros((128, 257), dtype=np.float32)
    gdA[np.arange(128), np.arange(128)] = float(N)
    gdB = np.zeros((128, 257), dtype=np.float32)
    gdB[np.arange(128), np.arange(128)] = float(N)
    corr_full = float(N) * (qa @ ka.T)                          # (256, 256) fp32
    corrb = np.zeros((256, 256), dtype=np.float32)
    for h in range(NH):
        corrb[h * D:(h + 1) * D, h * D:(h + 1) * D] = \
            corr_full[h * D:(h + 1) * D, h * D:(h + 1) * D]
    corrh = corrb.astype(np.float16)
    corrl = (corrb - corrh.astype(np.float32)).astype(np.float16)
    return {
        "qaT": qaT.astype(np.float16), "kaT": kaT.astype(np.float16),
        "wva": np.ascontiguousarray(wva, dtype=np.float16),
        "wpT": wpT.astype(np.float16),
        "boutc": np.ascontiguousarray(bout[:, None], dtype=np.float32),
        "bvc": np.ascontiguousarray(bv[:, None], dtype=np.float16),
        "qbr": np.ascontiguousarray(bq[None, :] * SCALE, dtype=np.float16),
        "kbr": np.ascontiguousarray(bk[None, :], dtype=np.float16),
        "ident": np.eye(128, dtype=np.float16),
        "gdA": gdA, "gdB": gdB, "corrh": corrh, "corrl": corrl,
    }


def kernel(x, Wqkv, bqkv, Wout, bout, num_heads):
    from concourse.bass_utils import run_bass_kernel_spmd

    assert int(num_heads) == NH
    x = np.ascontiguousarray(np.asarray(x), dtype=np.float32)
    shared = _host_prep(
        np.asarray(Wqkv, dtype=np.float32), np.asarray(bqkv, dtype=np.float32),
        np.asarray(Wout, dtype=np.float32), np.asarray(bout, dtype=np.float32))

    if "nc" not in _CACHE:
        _CACHE["nc"] = _build_real()
    nc = _CACHE["nc"]

    in_maps = [{"xb": np.ascontiguousarray(x[c].reshape(C, N)), **shared}
               for c in range(NCORES)]

    res = run_bass_kernel_spmd(nc, in_maps, core_ids=list(range(NCORES)),
                               trace=TRACE)
    LAST_RESULTS["exec_time_ns"] = res.exec_time_ns
    out = np.stack([res.results[c]["out"] for c in range(NCORES)])
    return out.reshape(B, C, H, W)
